# revision 63
# baseline (speedup 1.0000x reference)
"""Trainium2 Bass kernel for sigmoid-gated multi-head attention.

Reference computation (B=4, F=256, H=8, S=1024):
    qx  = q_input^T          (b, s, f)
    q   = qx @ Wq  -> (b, s, f, h)   [col fi*H + hi]
    k,v = kvx @ Wk / Wv
    attn = sigmoid(sqrt(F) * q.k)    per head
    wv   = attn @ v
    out  = relu(concat_heads(wv) @ Wz)   returned as (b, f, s)

Algebraic restructure (host-side weight folding):
    qkt_h = qx (Wq_h Wk_h^T) kvx^T = qx A_h kvx^T
    out   = relu(sum_h attn_h kvx (Wv_h Wz_h)) = relu(sum_h (attn_h kvx) N_h)
A_h and N_h are tiny 256x256 products computed on the host in fp32.
This removes the Q/K two-sided projection (only the cheap q-side
projection qa = A_h^T qin remains: 512 cols/core vs 1024) and removes
the Wz matmul entirely (N applied to raw = attn @ kvx: 256-contraction
instead of the 1024-wide u-projection).

Per-core MACs drop from 3.76G (baseline) to 2.684G = the perfect
8-way-sharding ideal, with zero collectives and zero duplicated
compute. PE floor 68.3us at fp16 rate.

Sharding: 8 cores = 4 batches x 2 query-sequence halves; per-core
outputs are disjoint slices of the final output.

Per head (all matmuls [128 x (2|8 chained) x 512], fp16 operands,
fp32 PSUM):
    qa   (fk 2x128, i 512) = A_h^T @ qin           4 mm   (pipelined 1 head ahead)
    qktT (j 8x128, i 512)  = kvin^T_slice @ qa    16 mm -> sigmoid(16x) -> atn
    rawT (fk 2x128, i 512) = kvx^T_slice... = sum_jb kvt_slice^T @ atn
                                                  16 mm (2 interleaved chains)
    outT (fo 2x128, i 512) += N_h^T @ rawT         4 mm  (persistent PSUM accum)
Engines: vector = PSUM->SBUF casts, scalar(ACT) = sigmoids + one qa
cast, gpsimd = weight/kvt DMA triggers, sync = qin/out DMA triggers.
"""

import os
import sys

sys.path.insert(0, "/opt/trn_rl_repo")

import numpy as np

B, F, H, S = 4, 256, 8, 1024
HALF = S // 2  # query columns per core
NCORES = 8
P = 128  # partitions

_cache = {}


def _build():
    import concourse.mybir as mybir
    import concourse.tile as tile
    from concourse import bacc

    dt = mybir.dt
    f32 = dt.float32
    f16 = dt.float16
    AF = mybir.ActivationFunctionType

    nc = bacc.Bacc(None, target_bir_lowering=False)

    # all partition-major: [P, ...] with per-partition lines contiguous
    qin_d = nc.dram_tensor("qin", [P, 2, HALF], f16, kind="ExternalInput")
    kvin_d = nc.dram_tensor("kvin", [P, 2, S], f16, kind="ExternalInput")
    # kvx in transposed layout [j(8x128 part), f] for the raw matmul
    kvt_d = nc.dram_tensor("kvt", [P, 8, F], f16, kind="ExternalInput")
    # per head: [slot 0=A_h rows g | slot 1=N_h rows f][c chunk][col].
    # Head-major so each per-head DMA is one fully contiguous 256KB block
    # (a [P, H, ...] layout makes the reads 2KB-strided and measurably
    # slower under 8-core HBM contention).
    w_d = nc.dram_tensor("w", [H, P, 2, 2, F], f16, kind="ExternalInput")
    out_d = nc.dram_tensor("out", [P, 2, HALF], f16, kind="ExternalOutput")

    with tile.TileContext(nc) as tc:
        with (
            tc.tile_pool(name="io", bufs=1) as io_pool,
            tc.tile_pool(name="wts", bufs=2) as w_pool,
            tc.tile_pool(name="qa", bufs=2) as qa_pool,
            tc.tile_pool(name="raw", bufs=2) as raw_pool,
            tc.tile_pool(name="attn", bufs=2) as attn_pool,
            tc.tile_pool(name="ps", bufs=6, space="PSUM") as ps_pool,
            tc.tile_pool(name="ops", bufs=1, space="PSUM") as out_ps_pool,
        ):
            # PE pre-warm: fine-grained (~106ns) dummy matmuls on a zeroed
            # bf16 tile bridge the gap until the first input DMAs land
            # (~13.5us; 8 cores share HBM) while keeping the PE clock
            # ramped -- an idle gap here costs a ~1-2.5us clock-ramp tax.
            nwarm = int(os.environ.get("ATTN_NWARM", "50"))
            if nwarm:
                warm = io_pool.tile([P, HALF], dt.bfloat16, tag="warm")
                nc.vector.memset(warm[:], 0.0)
                wps = [
                    ps_pool.tile([P, HALF], f32, tag="ps", name=f"wps{i}")
                    for i in range(2)
                ]
                for i in range(nwarm):
                    nc.tensor.matmul(
                        wps[i % 2][:, :P], warm[:, :P], warm[:, :P],
                        start=True, stop=True,
                    )

            qin = io_pool.tile([P, 2, HALF], f16, tag="qin")
            kvin = [
                io_pool.tile([P, S], f16, tag=f"kvin{c}", name=f"kvin{c}")
                for c in range(2)
            ]
            kvt = io_pool.tile([P, 8, F], f16, tag="kvt")

            # qin on the SP ring (first matmul input), kvin on the ACT
            # ring, w0/w1 + kvt on the gpsimd ring. Weights for heads 2+
            # are fetched inside the loop from the bufs=2 pool: the
            # buffer-reuse dependency holds each DMA until head h's
            # weights are consumed, keeping them out of the contended
            # front window (8 cores share HBM; the front set lands ~13us).
            ws = [None] * H

            def fetch_w(h):
                ws[h] = w_pool.tile([P, 2, 2, F], f16, tag="w", name=f"w{h}")
                nc.gpsimd.dma_start(ws[h][:], w_d[h])

            nc.sync.dma_start(qin[:], qin_d[:])
            nc.scalar.dma_start(kvin[0][:], kvin_d[:, 0])
            nc.scalar.dma_start(kvin[1][:], kvin_d[:, 1])
            fetch_w(0)
            fetch_w(1)
            nc.gpsimd.dma_start(kvt[:], kvt_d[:])

            # persistent accumulator for the folded output projection: 2 banks
            out_ps = out_ps_pool.tile([P, 2, HALF], f32, tag="out_ps")

            def qa_proj(h):
                """Emit q-side projection qa = A_h^T @ qin for head h."""
                qa = qa_pool.tile([P, 2, HALF], f16, tag="qa", name=f"qa{h}")
                for t in range(2):
                    ps = ps_pool.tile([P, HALF], f32, tag="ps", name=f"psqa{h}{t}")
                    for c in range(2):
                        nc.tensor.matmul(
                            ps[:],
                            ws[h][:, 0, c, P * t : P * (t + 1)],
                            qin[:, c, :],
                            start=(c == 0),
                            stop=(c == 1),
                        )
                    # split casts across engines to halve qa latency
                    if t == 0:
                        nc.vector.tensor_copy(qa[:, t, :], ps[:])
                    else:
                        nc.scalar.activation(qa[:, t, :], ps[:], AF.Copy)
                return qa

            qa_next = qa_proj(0)
            for h in range(H):
                qa = qa_next

                # qktT (j 8x128, i 512) = kvin_slice^T @ qa; sigmoid on ACT
                atn = attn_pool.tile([P, 8, HALF], f16, tag="atn", name=f"atn{h}")
                for jb in range(8):
                    ps = ps_pool.tile([P, HALF], f32, tag="ps")
                    for c in range(2):
                        nc.tensor.matmul(
                            ps[:],
                            kvin[c][:, P * jb : P * (jb + 1)],
                            qa[:, c, :],
                            start=(c == 0),
                            stop=(c == 1),
                        )
                    nc.scalar.activation(atn[:, jb, :], ps[:], AF.Sigmoid, scale=16.0)

                # fetch head h+2's weights (held by the pool-reuse dep
                # until head h's weights are consumed)
                if h + 2 < H:
                    fetch_w(h + 2)

                # rawT (fk 2x128, i 512) = sum_jb kvt_slice^T @ atn_jb.
                # Two chains (fk chunks) interleaved per jb so the PE
                # consumes each sigmoid output ~2x later than a straight
                # chain would -> no stall on the ACT engine's latency.
                raw = raw_pool.tile([P, 2, HALF], f16, tag="raw", name=f"raw{h}")
                rps = [
                    ps_pool.tile([P, HALF], f32, tag="ps", name=f"psr{h}{t}")
                    for t in range(2)
                ]
                if h < H - 1:
                    seq = [(jb, t) for jb in range(8) for t in range(2)]
                else:
                    # last head: finish the t=0 chain 2 matmuls early so its
                    # cast overlaps the t=1 chain tail and the final N-apply
                    # starts sooner (nothing else covers that latency here).
                    seq = [(jb, t) for jb in range(6) for t in range(2)]
                    seq += [(6, 0), (7, 0), (6, 1), (7, 1)]
                for jb, t in seq:
                    nc.tensor.matmul(
                        rps[t][:],
                        kvt[:, jb, P * t : P * (t + 1)],
                        atn[:, jb, :],
                        start=(jb == 0),
                        stop=(jb == 7),
                    )
                # both casts on vector: scalar is sigmoid-backlogged here,
                # and the c-outer N-apply below tolerates chunk 1 arriving
                # a full matmul-pair later than chunk 0.
                nc.vector.tensor_copy(raw[:, 0, :], rps[0][:])
                nc.vector.tensor_copy(raw[:, 1, :], rps[1][:])

                # software-pipeline: next head's qa projection here so the
                # PE has dependency-free work across the head boundary and
                # the qa casts have a full phase of latency cover.
                if h + 1 < H:
                    qa_next = qa_proj(h + 1)

                # outT (fo 2x128, i 512) += N_h^T @ rawT, persistent accum.
                # c-outer order: the first two matmuls only need raw chunk 0,
                # so they are not gated by the second (slower-engine) cast.
                last = h == H - 1
                for c in range(2):
                    for t2 in range(2):
                        nc.tensor.matmul(
                            out_ps[:, t2, :],
                            ws[h][:, 1, c, P * t2 : P * (t2 + 1)],
                            raw[:, c, :],
                            start=(h == 0 and c == 0),
                            stop=(last and c == 1),
                        )

            # tail: the two fo-chunks finish 1 matmul apart; run their
            # ReLUs on different engines in parallel, and issue the two
            # output-DMA triggers from different sequencers so the ~0.6us
            # DIRECT2D costs overlap.
            out_sb = io_pool.tile([P, 2, HALF], f16, tag="out_sb")
            nc.vector.tensor_relu(out_sb[:, 0, :], out_ps[:, 0, :])
            nc.sync.dma_start(out_d[:, 0], out_sb[:, 0, :])
            nc.scalar.activation(out_sb[:, 1, :], out_ps[:, 1, :], AF.Relu)
            nc.sync.dma_start(out_d[:, 1], out_sb[:, 1, :])

    nc.compile()
    return nc


def _get_nc():
    if "nc" not in _cache:
        _cache["nc"] = _build()
    return _cache["nc"]


def _make_in_maps(inputs):
    q_input = np.asarray(inputs["q_input"], dtype=np.float32)
    kv_input = np.asarray(inputs["kv_input"], dtype=np.float32)
    Wq4 = np.asarray(inputs["Wq"], dtype=np.float32).reshape(F, F, H)
    Wk4 = np.asarray(inputs["Wk"], dtype=np.float32).reshape(F, F, H)
    Wv4 = np.asarray(inputs["Wv"], dtype=np.float32).reshape(F, F, H)
    Wz3 = np.asarray(inputs["Wz"], dtype=np.float32).reshape(F, H, F)

    # host-side folds in fp32: A_h = Wq_h Wk_h^T, N_h = Wv_h Wz_h
    A = np.einsum("gdh,fdh->hgf", Wq4, Wk4, optimize=True)  # [H, g, fk]
    N = np.einsum("fdh,dho->hfo", Wv4, Wz3, optimize=True)  # [H, fk, fo]
    # w[h, p, 0, c, fk] = A[h, c*128+p, fk]; w[h, p, 1, c, fo] = N[h, ...]
    WALL = np.stack([A.reshape(H, 2, P, F), N.reshape(H, 2, P, F)], axis=1)
    WALL = np.ascontiguousarray(
        WALL.transpose(0, 3, 1, 2, 4), dtype=np.float16
    )  # [H, P, 2, 2, F]

    in_maps = []
    for core in range(NCORES):
        b, half = divmod(core, 2)
        qb = q_input[b].reshape(2, P, S)
        qin = np.ascontiguousarray(
            qb[:, :, half * HALF : (half + 1) * HALF].transpose(1, 0, 2),
            dtype=np.float16,
        )
        kvin = np.ascontiguousarray(
            kv_input[b].reshape(2, P, S).transpose(1, 0, 2), dtype=np.float16
        )
        # kvt[p, jb, f] = kv_input[b][f, jb*128+p]
        kvt = np.ascontiguousarray(
            kv_input[b].T.reshape(8, P, F).transpose(1, 0, 2), dtype=np.float16
        )
        in_maps.append({"qin": qin, "kvin": kvin, "kvt": kvt, "w": WALL})
    return in_maps


def kernel(q_input, kv_input, Wq, Wk, Wv, Wz, **kw):
    from concourse.bass_utils import run_bass_kernel_spmd

    nc = _get_nc()
    in_maps = _make_in_maps(
        {
            "q_input": q_input,
            "kv_input": kv_input,
            "Wq": Wq,
            "Wk": Wk,
            "Wv": Wv,
            "Wz": Wz,
        }
    )

    res = run_bass_kernel_spmd(nc, in_maps, core_ids=list(range(NCORES)))

    out = np.empty((B, F, S), dtype=np.float32)
    for c in range(NCORES):
        b, half = divmod(c, 2)
        # out dram [p, chunk, i] -> out[b, chunk*128+p, half*512+i]
        o = np.asarray(res.results[c]["out"], dtype=np.float32)  # (P, 2, HALF)
        out[b, :, half * HALF : (half + 1) * HALF] = o.transpose(1, 0, 2).reshape(
            F, HALF
        )
    return out
